# revision 4
# baseline (speedup 1.0000x reference)
"""MixTreeLSTMCell Trainium2 kernel (8 NeuronCores, SPMD).

Strategy
--------
The cell evaluates one of two branches per node depending on t in {0,1}.
Computing both branches for every node doubles the matmul flops and makes
the kernel PE-bound (~2x the memory roofline).  Instead the host
partitions the nodes by type and hands every core an equal number of
type-0 and type-1 nodes (padded up to a 512-node tile multiple), so the
device program has two static segments and no per-node select.

All matmul operands are laid out feature-major on the host (x^T, h^T and
the transposed weight matrices), so the device does no on-chip
transposes; matmuls run as float32r (full PE rate at free dim >= 256,
~1e-4 relative error).  Gates are drained from PSUM by the scalar engine
with the bias applied natively per partition; the vector engine runs the
remaining elementwise chain.  Outputs are produced feature-major and
un-permuted/transposed back on the host.
"""

from contextlib import ExitStack

import numpy as np

import concourse.bacc as bacc
import concourse.tile as tile
from concourse import mybir
from concourse import bass_utils

F32 = mybir.dt.float32
F32R = mybir.dt.float32r

N_NODES = 131072
X = 300
H = 256
CORES = 8
TILE_N = 512  # nodes per device tile (matmul free dim)

# Set by test harness to profile; LAST_EXEC_NS is filled after each run.
TRACE = False
LAST_EXEC_NS = None

_PROGRAM_CACHE = {}


def _round_up(v, m):
    return (v + m - 1) // m * m


def _build_program(T0, T1):
    """Trace + compile the SPMD program for T0 type-0 tiles and T1 type-1
    tiles of 512 nodes each (identical on all cores)."""
    key = (T0, T1)
    if key in _PROGRAM_CACHE:
        return _PROGRAM_CACHE[key]

    Nc = (T0 + T1) * TILE_N
    nc = bacc.Bacc("TRN2", target_bir_lowering=False, debug=False)

    xT = nc.dram_tensor("xT", [X, Nc], F32R, kind="ExternalInput").ap()
    hT = nc.dram_tensor("hT", [2 * H, Nc], F32R, kind="ExternalInput").ap()
    cT = nc.dram_tensor("cT", [2 * H, Nc], F32, kind="ExternalInput").ap()

    WnT = nc.dram_tensor("WnT", [X, 3 * H], F32R, kind="ExternalInput").ap()
    UnT = nc.dram_tensor("UnT", [2 * H, 3 * H], F32R, kind="ExternalInput").ap()
    UfwT = nc.dram_tensor("UfwT", [2 * H, 2 * H], F32R, kind="ExternalInput").ap()
    WsT = nc.dram_tensor("WsT", [X, 3 * H], F32R, kind="ExternalInput").ap()
    UsT = nc.dram_tensor("UsT", [H, 3 * H], F32R, kind="ExternalInput").ap()
    UfswT = nc.dram_tensor("UfswT", [H, H], F32R, kind="ExternalInput").ap()

    bias_n = nc.dram_tensor("bias_n", [128, 6], F32, kind="ExternalInput").ap()
    bias_fn = nc.dram_tensor("bias_fn", [128, 4], F32, kind="ExternalInput").ap()
    bias_s = nc.dram_tensor("bias_s", [128, 6], F32, kind="ExternalInput").ap()
    bias_fs = nc.dram_tensor("bias_fs", [128, 2], F32, kind="ExternalInput").ap()

    hOT = nc.dram_tensor("hOT", [H, Nc], F32, kind="ExternalOutput").ap()
    cOT = nc.dram_tensor("cOT", [H, Nc], F32, kind="ExternalOutput").ap()

    # feature-major [p, ko, n] views of the DRAM activations
    hT_v = hT.rearrange("(ko p) n -> p ko n", p=128)
    cT_v = cT.rearrange("(ko p) n -> p ko n", p=128)
    hOT_v = hOT.rearrange("(ko p) n -> p ko n", p=128)
    cOT_v = cOT.rearrange("(ko p) n -> p ko n", p=128)
    UnT_v = UnT.rearrange("(ko p) m -> p ko m", p=128)
    UfwT_v = UfwT.rearrange("(ko p) m -> p ko m", p=128)
    UsT_v = UsT.rearrange("(ko p) m -> p ko m", p=128)
    UfswT_v = UfswT.rearrange("(ko p) m -> p ko m", p=128)

    SIG = mybir.ActivationFunctionType.Sigmoid
    TANH = mybir.ActivationFunctionType.Tanh

    with tile.TileContext(nc) as tc, ExitStack() as stack:
        wp = stack.enter_context(tc.tile_pool(name="w", bufs=1))
        io = stack.enter_context(tc.tile_pool(name="io", bufs=2))
        mid = stack.enter_context(tc.tile_pool(name="mid", bufs=2))
        psp = stack.enter_context(tc.tile_pool(name="ps", bufs=4, space="PSUM"))

        # --- resident weights ---
        Wn_sb = wp.tile([128, 3, 3 * H], F32R)
        Ws_sb = wp.tile([128, 3, 3 * H], F32R)
        for k in range(2):
            nc.sync.dma_start(out=Wn_sb[:, k, :], in_=WnT[128 * k : 128 * (k + 1), :])
            nc.sync.dma_start(out=Ws_sb[:, k, :], in_=WsT[128 * k : 128 * (k + 1), :])
        nc.sync.dma_start(out=Wn_sb[: X - 256, 2, :], in_=WnT[256:X, :])
        nc.sync.dma_start(out=Ws_sb[: X - 256, 2, :], in_=WsT[256:X, :])
        Un_sb = wp.tile([128, 4, 3 * H], F32R)
        nc.sync.dma_start(out=Un_sb, in_=UnT_v)
        Ufw_sb = wp.tile([128, 4, 2 * H], F32R)
        nc.sync.dma_start(out=Ufw_sb, in_=UfwT_v)
        Us_sb = wp.tile([128, 2, 3 * H], F32R)
        nc.sync.dma_start(out=Us_sb, in_=UsT_v)
        Ufsw_sb = wp.tile([128, 2, H], F32R)
        nc.sync.dma_start(out=Ufsw_sb, in_=UfswT_v)
        bn_sb = wp.tile([128, 6], F32)
        nc.sync.dma_start(out=bn_sb, in_=bias_n)
        bfn_sb = wp.tile([128, 4], F32)
        nc.sync.dma_start(out=bfn_sb, in_=bias_fn)
        bs_sb = wp.tile([128, 6], F32)
        nc.sync.dma_start(out=bs_sb, in_=bias_s)
        bfs_sb = wp.tile([128, 2], F32)
        nc.sync.dma_start(out=bfs_sb, in_=bias_fs)

        def x_matmuls(ps, xt, W_sb, m, start):
            # x-part of the iou accumulation: 3 k-tiles (128/128/44)
            ms = slice(128 * m, 128 * (m + 1))
            nc.tensor.matmul(ps, W_sb[:, 0, ms], xt[:, 0, :], start=start, stop=False)
            nc.tensor.matmul(ps, W_sb[:, 1, ms], xt[:, 1, :], start=False, stop=False)
            nc.tensor.matmul(
                ps, W_sb[: X - 256, 2, ms], xt[: X - 256, 2, :], start=False, stop=False
            )

        def do_tile(br, n0):
            xt = io.tile([128, 3, TILE_N], F32R, tag="xt")
            for k in range(2):
                nc.sync.dma_start(
                    out=xt[:, k, :], in_=xT[128 * k : 128 * (k + 1), n0 : n0 + TILE_N]
                )
            nc.sync.dma_start(out=xt[: X - 256, 2, :], in_=xT[256:X, n0 : n0 + TILE_N])
            ht = io.tile([128, 4, TILE_N], F32R, tag="ht")
            nc.sync.dma_start(out=ht, in_=hT_v[:, :, n0 : n0 + TILE_N])
            ct = io.tile([128, 4, TILE_N], F32, tag="ct")
            nc.sync.dma_start(out=ct, in_=cT_v[:, :, n0 : n0 + TILE_N])

            # --- forget gates f: [128, 4, TILE_N] = 512 features x nodes ---
            f = mid.tile([128, 4, TILE_N], F32, tag="f")
            if br == 0:
                for m in range(4):
                    ps = psp.tile([128, TILE_N], F32, tag="ps")
                    for k in range(4):
                        nc.tensor.matmul(
                            ps,
                            Ufw_sb[:, k, 128 * m : 128 * (m + 1)],
                            ht[:, k, :],
                            start=(k == 0),
                            stop=(k == 3),
                        )
                    nc.scalar.activation(
                        out=f[:, m, :], in_=ps, func=SIG, bias=bfn_sb[:, m : m + 1]
                    )
            else:
                for child in range(2):
                    for m in range(2):
                        ps = psp.tile([128, TILE_N], F32, tag="ps")
                        for k in range(2):
                            nc.tensor.matmul(
                                ps,
                                Ufsw_sb[:, k, 128 * m : 128 * (m + 1)],
                                ht[:, 2 * child + k, :],
                                start=(k == 0),
                                stop=(k == 1),
                            )
                        nc.scalar.activation(
                            out=f[:, 2 * child + m, :],
                            in_=ps,
                            func=SIG,
                            bias=bfs_sb[:, m : m + 1],
                        )

            # prod = f * c_child (in place), c_red = child0 + child1
            nc.vector.tensor_mul(out=f, in0=f, in1=ct)
            cred = mid.tile([128, 2, TILE_N], F32, tag="cred")
            nc.vector.tensor_add(out=cred, in0=f[:, 0:2, :], in1=f[:, 2:4, :])

            if br == 1:
                htild = mid.tile([128, 2, TILE_N], F32R, tag="htild")
                nc.vector.tensor_add(out=htild, in0=ht[:, 0:2, :], in1=ht[:, 2:4, :])

            # --- iou gates: 6 m-tiles -> sigmoid(i), sigmoid(o), tanh(u) ---
            gates = mid.tile([128, 6, TILE_N], F32, tag="gates")
            for m in range(6):
                ps = psp.tile([128, TILE_N], F32, tag="ps")
                if br == 0:
                    x_matmuls(ps, xt, Wn_sb, m, start=True)
                    for k in range(4):
                        nc.tensor.matmul(
                            ps,
                            Un_sb[:, k, 128 * m : 128 * (m + 1)],
                            ht[:, k, :],
                            start=False,
                            stop=(k == 3),
                        )
                    bias = bn_sb[:, m : m + 1]
                else:
                    x_matmuls(ps, xt, Ws_sb, m, start=True)
                    for k in range(2):
                        nc.tensor.matmul(
                            ps,
                            Us_sb[:, k, 128 * m : 128 * (m + 1)],
                            htild[:, k, :],
                            start=False,
                            stop=(k == 1),
                        )
                    bias = bs_sb[:, m : m + 1]
                nc.scalar.activation(
                    out=gates[:, m, :],
                    in_=ps,
                    func=TANH if m >= 4 else SIG,
                    bias=bias,
                )

            # c = sig(i)*tanh(u) + c_red ; h = sig(o)*tanh(c)
            cout = mid.tile([128, 2, TILE_N], F32, tag="cout")
            nc.vector.tensor_mul(out=cout, in0=gates[:, 0:2, :], in1=gates[:, 4:6, :])
            nc.vector.tensor_add(out=cout, in0=cout, in1=cred)
            tct = mid.tile([128, 2, TILE_N], F32, tag="tct")
            nc.scalar.activation(out=tct, in_=cout, func=TANH)
            hout = mid.tile([128, 2, TILE_N], F32, tag="hout")
            nc.vector.tensor_mul(out=hout, in0=gates[:, 2:4, :], in1=tct)

            nc.sync.dma_start(out=hOT_v[:, :, n0 : n0 + TILE_N], in_=hout)
            nc.sync.dma_start(out=cOT_v[:, :, n0 : n0 + TILE_N], in_=cout)

        for i in range(T0):
            do_tile(0, i * TILE_N)
        for i in range(T1):
            do_tile(1, T0 * TILE_N + i * TILE_N)

    nc.compile()
    _PROGRAM_CACHE[key] = nc
    return nc


def kernel(x, h_child, c_child, t, W_iou, U_iou, b_iou, U_f_w, U_f_b,
           W_iou_s, U_iou_s, b_iou_s, U_f_s_w, U_f_s_b):
    global LAST_EXEC_NS
    x = np.asarray(x, dtype=np.float32)
    h_child = np.asarray(h_child, dtype=np.float32)
    c_child = np.asarray(c_child, dtype=np.float32)
    t = np.asarray(t)
    n = x.shape[0]

    # --- host partition: equal per-core type counts, padded to tiles ---
    idx0 = np.flatnonzero(t == 0)
    idx1 = np.flatnonzero(t != 0)
    n0, n1 = len(idx0), len(idx1)

    def pad_split(idx, cnt):
        if cnt == 0:
            return np.zeros((CORES, 0), dtype=np.int64), 0
        per = _round_up(-(-cnt // CORES), TILE_N)
        padded = np.concatenate(
            [idx, np.full(CORES * per - cnt, idx[-1], dtype=idx.dtype)]
        )
        return padded.reshape(CORES, per).astype(np.int64), per

    chunks0, P0 = pad_split(idx0, n0)
    chunks1, P1 = pad_split(idx1, n1)
    T0, T1 = P0 // TILE_N, P1 // TILE_N
    Nc = P0 + P1

    nc = _build_program(T0, T1)

    # --- weights (shared across cores) ---
    hc2 = h_child.reshape(n, 2 * H)
    cc2 = c_child.reshape(n, 2 * H)

    def bias_tile(v, m):
        # [m*128] bias vector -> [128, m] per-partition layout
        return np.ascontiguousarray(
            np.asarray(v, np.float32).reshape(-1)[: 128 * m].reshape(m, 128).T
        )

    wmap = {
        "WnT": np.ascontiguousarray(np.asarray(W_iou, np.float32).T),
        "UnT": np.ascontiguousarray(np.asarray(U_iou, np.float32).T),
        "UfwT": np.ascontiguousarray(np.asarray(U_f_w, np.float32).T),
        "WsT": np.ascontiguousarray(np.asarray(W_iou_s, np.float32).T),
        "UsT": np.ascontiguousarray(np.asarray(U_iou_s, np.float32).T),
        "UfswT": np.ascontiguousarray(np.asarray(U_f_s_w, np.float32).T),
        "bias_n": bias_tile(b_iou, 6),
        "bias_fn": bias_tile(U_f_b, 4),
        "bias_s": bias_tile(b_iou_s, 6),
        "bias_fs": bias_tile(U_f_s_b, 2),
    }

    in_maps = []
    for i in range(CORES):
        I = np.concatenate([chunks0[i], chunks1[i]])
        m = dict(wmap)
        m["xT"] = np.ascontiguousarray(x[I].T)
        m["hT"] = np.ascontiguousarray(hc2[I].T)
        m["cT"] = np.ascontiguousarray(cc2[I].T)
        in_maps.append(m)

    res = bass_utils.run_bass_kernel_spmd(
        nc, in_maps, core_ids=list(range(CORES)), trace=TRACE
    )
    LAST_EXEC_NS = res.exec_time_ns

    # --- scatter back ---
    h_out = np.empty((n, H), dtype=np.float32)
    c_out = np.empty((n, H), dtype=np.float32)
    if n0:
        h0 = np.concatenate([res.results[i]["hOT"][:, :P0].T for i in range(CORES)])
        c0 = np.concatenate([res.results[i]["cOT"][:, :P0].T for i in range(CORES)])
        h_out[idx0] = h0[:n0]
        c_out[idx0] = c0[:n0]
    if n1:
        h1 = np.concatenate([res.results[i]["hOT"][:, P0:].T for i in range(CORES)])
        c1 = np.concatenate([res.results[i]["cOT"][:, P0:].T for i in range(CORES)])
        h_out[idx1] = h1[:n1]
        c_out[idx1] = c1[:n1]
    return h_out, c_out


# revision 6
# speedup vs baseline: 1.1829x; 1.1829x over previous
"""MixTreeLSTMCell Trainium2 kernel (8 NeuronCores, SPMD).

Strategy
--------
The cell evaluates one of two branches per node depending on t in {0,1}.
Computing both branches for every node doubles the matmul flops and makes
the kernel PE-bound (~2x the memory roofline).  Instead the host
partitions the nodes by type and hands every core an equal number of
type-0 and type-1 nodes (padded up to a 512-node tile multiple), so the
device program has two static segments and no per-node select.

All matmul operands are laid out feature-major on the host (x^T, h^T and
the transposed weight matrices) so the device does no on-chip transposes,
and are cast to fp16 (halves the HBM traffic; matmuls accumulate fp32 in
PSUM).  Gates are drained from PSUM by the scalar engine with the bias
applied natively per partition; the vector engine runs the remaining
elementwise chain in fp32.  Loads are issued in 2048-node macro tiles for
4 KiB-per-partition DMA runs, split across both HWDGE rings (sync/scalar)
with stores on SWDGE (gpsimd) for queue parallelism.  Outputs are
produced feature-major fp32 and un-permuted/transposed on the host.
"""

from contextlib import ExitStack

import numpy as np

import concourse.bacc as bacc
import concourse.tile as tile
from concourse import mybir
from concourse import bass_utils

F32 = mybir.dt.float32
FP16 = mybir.dt.float16
NP_FP16 = np.float16

N_NODES = 131072
X = 300
H = 256
CORES = 8
TILE_N = 512          # nodes per compute tile (matmul free dim)
MACRO = 4 * TILE_N    # nodes per DMA macro tile

# Set by test harness to profile; LAST_EXEC_NS is filled after each run.
TRACE = False
LAST_EXEC_NS = None

_PROGRAM_CACHE = {}


def _round_up(v, m):
    return (v + m - 1) // m * m


def _build_program(T0, T1):
    """Trace + compile the SPMD program for T0 type-0 tiles and T1 type-1
    tiles of 512 nodes each (identical on all cores)."""
    key = (T0, T1)
    if key in _PROGRAM_CACHE:
        return _PROGRAM_CACHE[key]

    Nc = (T0 + T1) * TILE_N
    nc = bacc.Bacc("TRN2", target_bir_lowering=False, debug=False)

    xT = nc.dram_tensor("xT", [X, Nc], FP16, kind="ExternalInput").ap()
    hT = nc.dram_tensor("hT", [2 * H, Nc], FP16, kind="ExternalInput").ap()
    cT = nc.dram_tensor("cT", [2 * H, Nc], FP16, kind="ExternalInput").ap()

    WnT = nc.dram_tensor("WnT", [X, 3 * H], FP16, kind="ExternalInput").ap()
    UnT = nc.dram_tensor("UnT", [2 * H, 3 * H], FP16, kind="ExternalInput").ap()
    UfwT = nc.dram_tensor("UfwT", [2 * H, 2 * H], FP16, kind="ExternalInput").ap()
    WsT = nc.dram_tensor("WsT", [X, 3 * H], FP16, kind="ExternalInput").ap()
    UsT = nc.dram_tensor("UsT", [H, 3 * H], FP16, kind="ExternalInput").ap()
    UfswT = nc.dram_tensor("UfswT", [H, H], FP16, kind="ExternalInput").ap()

    bias_n = nc.dram_tensor("bias_n", [128, 6], F32, kind="ExternalInput").ap()
    bias_fn = nc.dram_tensor("bias_fn", [128, 4], F32, kind="ExternalInput").ap()
    bias_s = nc.dram_tensor("bias_s", [128, 6], F32, kind="ExternalInput").ap()
    bias_fs = nc.dram_tensor("bias_fs", [128, 2], F32, kind="ExternalInput").ap()

    hOT = nc.dram_tensor("hOT", [H, Nc], F32, kind="ExternalOutput").ap()
    cOT = nc.dram_tensor("cOT", [H, Nc], F32, kind="ExternalOutput").ap()

    # feature-major [p, ko, n] views of the DRAM activations
    hT_v = hT.rearrange("(ko p) n -> p ko n", p=128)
    cT_v = cT.rearrange("(ko p) n -> p ko n", p=128)
    hOT_v = hOT.rearrange("(ko p) n -> p ko n", p=128)
    cOT_v = cOT.rearrange("(ko p) n -> p ko n", p=128)
    UnT_v = UnT.rearrange("(ko p) m -> p ko m", p=128)
    UfwT_v = UfwT.rearrange("(ko p) m -> p ko m", p=128)
    UsT_v = UsT.rearrange("(ko p) m -> p ko m", p=128)
    UfswT_v = UfswT.rearrange("(ko p) m -> p ko m", p=128)

    SIG = mybir.ActivationFunctionType.Sigmoid
    TANH = mybir.ActivationFunctionType.Tanh

    with tile.TileContext(nc) as tc, ExitStack() as stack:
        wp = stack.enter_context(tc.tile_pool(name="w", bufs=1))
        io = stack.enter_context(tc.tile_pool(name="io", bufs=2))
        mid = stack.enter_context(tc.tile_pool(name="mid", bufs=2))
        psp = stack.enter_context(tc.tile_pool(name="ps", bufs=4, space="PSUM"))

        # --- resident weights ---
        Wn_sb = wp.tile([128, 3, 3 * H], FP16)
        Ws_sb = wp.tile([128, 3, 3 * H], FP16)
        for k in range(2):
            nc.sync.dma_start(out=Wn_sb[:, k, :], in_=WnT[128 * k : 128 * (k + 1), :])
            nc.sync.dma_start(out=Ws_sb[:, k, :], in_=WsT[128 * k : 128 * (k + 1), :])
        nc.sync.dma_start(out=Wn_sb[: X - 256, 2, :], in_=WnT[256:X, :])
        nc.sync.dma_start(out=Ws_sb[: X - 256, 2, :], in_=WsT[256:X, :])
        Un_sb = wp.tile([128, 4, 3 * H], FP16)
        nc.sync.dma_start(out=Un_sb, in_=UnT_v)
        Ufw_sb = wp.tile([128, 4, 2 * H], FP16)
        nc.sync.dma_start(out=Ufw_sb, in_=UfwT_v)
        Us_sb = wp.tile([128, 2, 3 * H], FP16)
        nc.sync.dma_start(out=Us_sb, in_=UsT_v)
        Ufsw_sb = wp.tile([128, 2, H], FP16)
        nc.sync.dma_start(out=Ufsw_sb, in_=UfswT_v)
        bn_sb = wp.tile([128, 6], F32)
        nc.sync.dma_start(out=bn_sb, in_=bias_n)
        bfn_sb = wp.tile([128, 4], F32)
        nc.sync.dma_start(out=bfn_sb, in_=bias_fn)
        bs_sb = wp.tile([128, 6], F32)
        nc.sync.dma_start(out=bs_sb, in_=bias_s)
        bfs_sb = wp.tile([128, 2], F32)
        nc.sync.dma_start(out=bfs_sb, in_=bias_fs)

        def x_matmuls(ps, xt, j, W_sb, m, start):
            # x-part of the iou accumulation: 3 k-tiles (128/128/44)
            ms = slice(128 * m, 128 * (m + 1))
            ns = slice(j * TILE_N, (j + 1) * TILE_N)
            nc.tensor.matmul(ps, W_sb[:, 0, ms], xt[:, 0, ns], start=start, stop=False)
            nc.tensor.matmul(ps, W_sb[:, 1, ms], xt[:, 1, ns], start=False, stop=False)
            nc.tensor.matmul(
                ps, W_sb[: X - 256, 2, ms], xt[: X - 256, 2, ns], start=False, stop=False
            )

        def do_tile(br, xt, ht, ct, j, n0):
            """Process one 512-node tile; xt/ht/ct are MACRO tiles, j is the
            tile index inside the macro, n0 the node offset in DRAM."""
            ns = slice(j * TILE_N, (j + 1) * TILE_N)

            # --- forget gates f: [128, 4, TILE_N] = 512 features x nodes ---
            f = mid.tile([128, 4, TILE_N], F32, tag="f")
            if br == 0:
                for m in range(4):
                    ps = psp.tile([128, TILE_N], F32, tag="ps")
                    for k in range(4):
                        nc.tensor.matmul(
                            ps,
                            Ufw_sb[:, k, 128 * m : 128 * (m + 1)],
                            ht[:, k, ns],
                            start=(k == 0),
                            stop=(k == 3),
                        )
                    nc.scalar.activation(
                        out=f[:, m, :], in_=ps, func=SIG, bias=bfn_sb[:, m : m + 1]
                    )
            else:
                for child in range(2):
                    for m in range(2):
                        ps = psp.tile([128, TILE_N], F32, tag="ps")
                        for k in range(2):
                            nc.tensor.matmul(
                                ps,
                                Ufsw_sb[:, k, 128 * m : 128 * (m + 1)],
                                ht[:, 2 * child + k, ns],
                                start=(k == 0),
                                stop=(k == 1),
                            )
                        nc.scalar.activation(
                            out=f[:, 2 * child + m, :],
                            in_=ps,
                            func=SIG,
                            bias=bfs_sb[:, m : m + 1],
                        )

            # prod = f * c_child (in place), c_red = child0 + child1
            nc.vector.tensor_mul(out=f, in0=f, in1=ct[:, :, ns])
            cred = mid.tile([128, 2, TILE_N], F32, tag="cred")
            nc.vector.tensor_add(out=cred, in0=f[:, 0:2, :], in1=f[:, 2:4, :])

            if br == 1:
                htild = mid.tile([128, 2, TILE_N], FP16, tag="htild")
                nc.vector.tensor_add(out=htild, in0=ht[:, 0:2, ns], in1=ht[:, 2:4, ns])

            # --- iou gates: 6 m-tiles -> sigmoid(i), sigmoid(o), tanh(u) ---
            gates = mid.tile([128, 6, TILE_N], F32, tag="gates")
            for m in range(6):
                ps = psp.tile([128, TILE_N], F32, tag="ps")
                if br == 0:
                    x_matmuls(ps, xt, j, Wn_sb, m, start=True)
                    for k in range(4):
                        nc.tensor.matmul(
                            ps,
                            Un_sb[:, k, 128 * m : 128 * (m + 1)],
                            ht[:, k, ns],
                            start=False,
                            stop=(k == 3),
                        )
                    bias = bn_sb[:, m : m + 1]
                else:
                    x_matmuls(ps, xt, j, Ws_sb, m, start=True)
                    for k in range(2):
                        nc.tensor.matmul(
                            ps,
                            Us_sb[:, k, 128 * m : 128 * (m + 1)],
                            htild[:, k, :],
                            start=False,
                            stop=(k == 1),
                        )
                    bias = bs_sb[:, m : m + 1]
                nc.scalar.activation(
                    out=gates[:, m, :],
                    in_=ps,
                    func=TANH if m >= 4 else SIG,
                    bias=bias,
                )

            # c = sig(i)*tanh(u) + c_red ; h = sig(o)*tanh(c)
            cout = mid.tile([128, 2, TILE_N], F32, tag="cout")
            nc.vector.tensor_mul(out=cout, in0=gates[:, 0:2, :], in1=gates[:, 4:6, :])
            nc.vector.tensor_add(out=cout, in0=cout, in1=cred)
            tct = mid.tile([128, 2, TILE_N], F32, tag="tct")
            nc.scalar.activation(out=tct, in_=cout, func=TANH)
            hout = mid.tile([128, 2, TILE_N], F32, tag="hout")
            nc.vector.tensor_mul(out=hout, in0=gates[:, 2:4, :], in1=tct)

            nc.gpsimd.dma_start(out=hOT_v[:, :, n0 : n0 + TILE_N], in_=hout)
            nc.gpsimd.dma_start(out=cOT_v[:, :, n0 : n0 + TILE_N], in_=cout)

        # macro-tile loop: load 2048 nodes at a time, compute 4 tiles
        T = T0 + T1
        assert T0 % 4 == 0 and T1 % 4 == 0
        for g in range(T // 4):
            n0 = g * MACRO
            br = 0 if n0 < T0 * TILE_N else 1
            xt = io.tile([128, 3, MACRO], FP16, tag="xt")
            for k in range(2):
                nc.sync.dma_start(
                    out=xt[:, k, :], in_=xT[128 * k : 128 * (k + 1), n0 : n0 + MACRO]
                )
            nc.sync.dma_start(out=xt[: X - 256, 2, :], in_=xT[256:X, n0 : n0 + MACRO])
            ht = io.tile([128, 4, MACRO], FP16, tag="ht")
            nc.sync.dma_start(out=ht, in_=hT_v[:, :, n0 : n0 + MACRO])
            ct = io.tile([128, 4, MACRO], FP16, tag="ct")
            nc.scalar.dma_start(out=ct, in_=cT_v[:, :, n0 : n0 + MACRO])
            for j in range(4):
                do_tile(br, xt, ht, ct, j, n0 + j * TILE_N)

    nc.compile()
    _PROGRAM_CACHE[key] = nc
    return nc


def kernel(x, h_child, c_child, t, W_iou, U_iou, b_iou, U_f_w, U_f_b,
           W_iou_s, U_iou_s, b_iou_s, U_f_s_w, U_f_s_b):
    global LAST_EXEC_NS
    x = np.asarray(x, dtype=np.float32)
    h_child = np.asarray(h_child, dtype=np.float32)
    c_child = np.asarray(c_child, dtype=np.float32)
    t = np.asarray(t)
    n = x.shape[0]

    # --- host partition: equal per-core type counts, padded to macro tiles ---
    idx0 = np.flatnonzero(t == 0)
    idx1 = np.flatnonzero(t != 0)
    n0, n1 = len(idx0), len(idx1)

    def pad_split(idx, cnt):
        if cnt == 0:
            return np.zeros((CORES, 0), dtype=np.int64), 0
        per = _round_up(-(-cnt // CORES), MACRO)
        padded = np.concatenate(
            [idx, np.full(CORES * per - cnt, idx[-1], dtype=idx.dtype)]
        )
        return padded.reshape(CORES, per).astype(np.int64), per

    chunks0, P0 = pad_split(idx0, n0)
    chunks1, P1 = pad_split(idx1, n1)
    T0, T1 = P0 // TILE_N, P1 // TILE_N

    nc = _build_program(T0, T1)

    # --- weights (shared across cores) ---
    hc2 = h_child.reshape(n, 2 * H)
    cc2 = c_child.reshape(n, 2 * H)

    def bias_tile(v, m):
        # [m*128] bias vector -> [128, m] per-partition layout
        return np.ascontiguousarray(
            np.asarray(v, np.float32).reshape(-1)[: 128 * m].reshape(m, 128).T
        )

    wmap = {
        "WnT": np.ascontiguousarray(np.asarray(W_iou, np.float32).T).astype(NP_FP16),
        "UnT": np.ascontiguousarray(np.asarray(U_iou, np.float32).T).astype(NP_FP16),
        "UfwT": np.ascontiguousarray(np.asarray(U_f_w, np.float32).T).astype(NP_FP16),
        "WsT": np.ascontiguousarray(np.asarray(W_iou_s, np.float32).T).astype(NP_FP16),
        "UsT": np.ascontiguousarray(np.asarray(U_iou_s, np.float32).T).astype(NP_FP16),
        "UfswT": np.ascontiguousarray(np.asarray(U_f_s_w, np.float32).T).astype(NP_FP16),
        "bias_n": bias_tile(b_iou, 6),
        "bias_fn": bias_tile(U_f_b, 4),
        "bias_s": bias_tile(b_iou_s, 6),
        "bias_fs": bias_tile(U_f_s_b, 2),
    }

    in_maps = []
    for i in range(CORES):
        I = np.concatenate([chunks0[i], chunks1[i]])
        m = dict(wmap)
        m["xT"] = x[I].T.astype(NP_FP16)
        m["hT"] = hc2[I].T.astype(NP_FP16)
        m["cT"] = cc2[I].T.astype(NP_FP16)
        in_maps.append(m)

    res = bass_utils.run_bass_kernel_spmd(
        nc, in_maps, core_ids=list(range(CORES)), trace=TRACE
    )
    LAST_EXEC_NS = res.exec_time_ns

    # --- scatter back ---
    h_out = np.empty((n, H), dtype=np.float32)
    c_out = np.empty((n, H), dtype=np.float32)
    if n0:
        h0 = np.concatenate([res.results[i]["hOT"][:, :P0].T for i in range(CORES)])
        c0 = np.concatenate([res.results[i]["cOT"][:, :P0].T for i in range(CORES)])
        h_out[idx0] = h0[:n0]
        c_out[idx0] = c0[:n0]
    if n1:
        h1 = np.concatenate([res.results[i]["hOT"][:, P0:].T for i in range(CORES)])
        c1 = np.concatenate([res.results[i]["cOT"][:, P0:].T for i in range(CORES)])
        h_out[idx1] = h1[:n1]
        c_out[idx1] = c1[:n1]
    return h_out, c_out
